# revision 11
# baseline (speedup 1.0000x reference)
"""CQAttention Trainium2 kernel (8-core data parallel), v4.

Math (per example):
    S[i,j] = C@w_c [i] + Q@w_q [j] + (C*w_mul)@Q^T [i,j] + bias
    S1 = softmax_j(where(Qmask==0, -1e9, S))
    S2 = softmax_i(where(Cmask==0, -1e9, S))
    A  = S1 @ Q
    Bm = S1 @ S2^T @ C
    out = concat([C, A, C*A, C*Bm], axis=-1)

Key identities:
  - softmax shift-invariance: `bias` drops out; per-row offsets cancel in
    S1; per-column offsets (s1_j + qneg_j) cancel in S2.
  - One exp serves both softmaxes: E^T[j,i] = exp(s2 + s0_i + s1_j + qneg_j)
    computed j-major; the i-major copy needed by the S2 contraction is a PE
    transpose of E^T (exact, no second exp). Masked-j columns of the
    transposed tiles are all-zero, so the S2 denominator gets a +1e-30
    guard; those T' rows are then zero and carry zero weight in A|Bm.
  - The C-side mask is folded into the host-packed traw rhs [cm*C | cm].
  - The A|Bm matmul per tile is 256 wide so TWO tiles share a PSUM bank
    and drain with ONE 512-col cast; the S1 denominator r is NOT a matmul
    column but a vector free-dim reduction over the transposed exp tiles.
    Raw Araw|Bmraw|r go out in bf16; the host divides during assembly.

Device outputs only Araw|Bmraw|r; the host assembles [C, A, C*A, C*Bm].
Loads are per-example, ordered so the critical path (consts+QT, CT0) hits
the DMA queues first; compute starts ~8.5us. Triggers are spread over
sync (CST+CT02+cub01+outputs), scalar (first CUBs) and gpsimd (Q*, loop
CT/CUB). Postprocessing is balanced: scalar = exp + 2 pair casts, vector
= exp-transpose copy + r-reduce + T' + 2 pair casts, gpsimd = qm + r cast.
"""

import os
import sys
from contextlib import ExitStack

import ml_dtypes
import numpy as np

for _p in ("/opt/trn_rl_repo", "/root/.axon_site/_ro/trn_rl_repo"):
    if os.path.isdir(_p) and _p not in sys.path:
        sys.path.append(_p)

import concourse.bass as bass
import concourse.tile as tile
from concourse import bacc, mybir
from concourse.bass import ds, ts
from concourse.bass_utils import run_bass_kernel_spmd
from concourse.masks import make_identity

F32 = mybir.dt.float32
FP16 = mybir.dt.float16
BF16 = mybir.dt.bfloat16
AF = mybir.ActivationFunctionType
ALU = mybir.AluOpType
AX = mybir.AxisListType

N_CORES = 8
B, LC, LQ, D = 64, 1024, 128, 128
B_LOC = B // N_CORES  # 8 examples per core
NT = LC // 128  # 8 Lc tiles of 128
OW = NT * 256 + NT  # per-example out cols: 8 tiles of [A|Bm] + 8 r cols


def _build_graph():
    nc = bacc.Bacc("TRN2", target_bir_lowering=False, debug=False)

    CT8 = nc.dram_tensor("CT8", [B_LOC, 128, LC], FP16, kind="ExternalInput").ap()
    CUB8 = nc.dram_tensor("CUB8", [B_LOC, 128, NT * 129], BF16, kind="ExternalInput").ap()
    QTP = nc.dram_tensor("QTP", [128, B_LOC, LQ], FP16, kind="ExternalInput").ap()
    QB1 = nc.dram_tensor("QB1", [128, B_LOC, 128], BF16, kind="ExternalInput").ap()
    # packed consts: 0 w_mul | 1 w_c | 2:10 qneg | 10:12 w_q
    CST = nc.dram_tensor("CST", [128, 12], F32, kind="ExternalInput").ap()
    # out, p-major: [e, p, t*256 + (Araw(128)|Bmraw(128))], then r at 2048+t
    OUT = nc.dram_tensor("OUT", [B_LOC, 128, OW], BF16, kind="ExternalOutput").ap()

    with tile.TileContext(nc) as tc:
        with ExitStack() as ctx:
            ep = ctx.enter_context

            const = ep(tc.tile_pool(name="const", bufs=1))
            p_qt = ep(tc.tile_pool(name="qt", bufs=1))
            p_arhs = ep(tc.tile_pool(name="arhs", bufs=1))
            p_ct = ep(tc.tile_pool(name="ct", bufs=B_LOC))
            p_cub = ep(tc.tile_pool(name="cub", bufs=B_LOC))
            p_qm = ep(tc.tile_pool(name="qm", bufs=B_LOC))
            p_eq = ep(tc.tile_pool(name="eq", bufs=3))
            p_ecp = ep(tc.tile_pool(name="ecp", bufs=2))
            p_stg = ep(tc.tile_pool(name="stg", bufs=3))
            p_rc = ep(tc.tile_pool(name="rc", bufs=3))
            p_small = ep(tc.tile_pool(name="small", bufs=40))

            # PSUM: exactly 8 banks, every tile bank-sized
            pp_e1 = ep(tc.tile_pool(name="pp_e1", bufs=1, space="PSUM"))
            pp_tp = ep(tc.tile_pool(name="pp_tp", bufs=1, space="PSUM"))
            pp_ts = ep(tc.tile_pool(name="pp_ts", bufs=1, space="PSUM"))
            pp_ab = ep(tc.tile_pool(name="pp_ab", bufs=2, space="PSUM"))

            # ---- tiles ----
            ct_sb = [p_ct.tile([128, LC], FP16, tag="ct", name=f"ct{e}") for e in range(B_LOC)]
            cub_sb = [p_cub.tile([128, NT * 129], BF16, tag="cub", name=f"cub{e}") for e in range(B_LOC)]
            qt_all = p_qt.tile([128, B_LOC, LQ], FP16, tag="qt")
            arhs = p_arhs.tile([128, B_LOC, 256], BF16, tag="arhs")
            cst = const.tile([128, 12], F32)
            wq_sb = const.tile([D, 2], FP16)
            warm_w = const.tile([128, 512], BF16)
            ident = const.tile([128, 128], BF16)
            bias1 = const.tile([LQ, B_LOC], F32)

            # ---- head: critical loads first (QT, CST, CT0), then the rest
            nc.gpsimd.memset(warm_w, 1.0)
            nc.sync.dma_start(qt_all, QTP)
            nc.sync.dma_start(cst, CST)
            nc.sync.dma_start(ct_sb[0], CT8[0])
            nc.scalar.dma_start(cub_sb[0], CUB8[0])
            nc.scalar.dma_start(arhs[:, :, 0:128], QB1)
            nc.scalar.dma_start(cub_sb[1], CUB8[1])
            nc.sync.dma_start(ct_sb[1], CT8[1])
            nc.sync.dma_start(ct_sb[2], CT8[2])

            make_identity(nc, ident)
            nc.gpsimd.tensor_copy(wq_sb, cst[:, 10:12])

            # PE warmup during the load head (HAM mode settle)
            for i in range(4):
                warm_ps = pp_e1.tile([128, 512], F32, tag="pe1", name=f"warm{i}")
                nc.tensor.matmul(warm_ps, lhsT=warm_w[:, 0:128], rhs=warm_w)

            # Qm' = w_mul * Q^T + w_c  (first two on vector, rest gpsimd)
            qm_sb = []
            for e in range(B_LOC):
                qm = p_qm.tile([128, LQ], FP16, tag="qm", name=f"qm{e}")
                eng = nc.vector if e < 2 else nc.gpsimd
                eng.tensor_scalar(
                    qm, qt_all[:, e, :], cst[:, 0:1], cst[:, 1:2],
                    op0=ALU.mult, op1=ALU.add,
                )
                qm_sb.append(qm)

            # s1_j = Q@w_q, all examples, one PSUM bank; bias1 = s1 + qneg
            s1_ps = pp_ts.tile([128, 512], F32, tag="pts", name="s1")
            s1_v = s1_ps[:, 0 : 2 * B_LOC].rearrange("p (e k) -> p e k", k=2)
            for e in range(B_LOC):
                nc.tensor.matmul(s1_v[:, e, :], lhsT=qt_all[:, e, :], rhs=wq_sb)
            nc.vector.tensor_add(
                bias1, s1_v[:, :, 0:1].rearrange("p e k -> p (e k)"), cst[:, 2:10]
            )

            eq_sb = [None] * B_LOC
            ecp_sb = [None] * B_LOC
            rc_sb = [None] * B_LOC

            # ---- pipeline stages ----
            def emit_e1(e):
                eq = p_eq.tile([128, LC], BF16, tag="eq", name=f"eq{e}")
                ps = pp_e1.tile([128, 2, 512], F32, tag="pe1", name=f"e1_{e}")
                for h in range(2):
                    nc.tensor.matmul(
                        ps[:, h, :], lhsT=qm_sb[e], rhs=ct_sb[e][:, ts(h, 512)]
                    )
                nc.scalar.activation(
                    eq, ps.rearrange("p h x -> p (h x)"), func=AF.Exp,
                    bias=bias1[:, e : e + 1], scale=1.0,
                )
                eq_sb[e] = eq

            def emit_tp(e):
                tpps = pp_tp.tile([128, NT, 128], BF16, tag="ptp", name=f"tp{e}")
                for t in range(NT):
                    nc.tensor.transpose(tpps[:, t, :], eq_sb[e][:, ts(t, 128)], ident)
                ecp = p_ecp.tile([128, NT, 128], BF16, tag="ecp", name=f"ecp{e}")
                nc.scalar.activation(
                    ecp.rearrange("p t x -> p (t x)"),
                    tpps.rearrange("p t x -> p (t x)"), func=AF.Copy,
                )
                rc = p_rc.tile([128, NT], F32, tag="rc", name=f"rc{e}")
                nc.vector.tensor_reduce(rc, ecp, axis=AX.X, op=ALU.add)
                ecp_sb[e] = ecp
                rc_sb[e] = rc

            def emit_traw(e):
                trps = pp_ts.tile([128, 512], F32, tag="pts", name=f"traw{e}")
                for t in range(NT):
                    nc.tensor.matmul(
                        trps[:, 0:129],
                        lhsT=ecp_sb[e][:, t, :],
                        rhs=cub_sb[e][:, ds(129 * t, 129)],
                        start=(t == 0),
                        stop=(t == NT - 1),
                    )
                ceps = p_small.tile([128, 1], F32, tag="small", name=f"ceps{e}")
                nc.scalar.activation(
                    ceps, trps[:, 128:129], func=AF.Copy, bias=1e-30
                )
                cinv = p_small.tile([128, 1], F32, tag="small", name=f"cinv{e}")
                nc.vector.reciprocal(cinv, ceps)
                nc.vector.tensor_scalar(
                    arhs[:, e, 128:256], trps[:, 0:128], cinv, None, op0=ALU.mult
                )

            def emit_abm(e):
                stg = p_stg.tile([128, OW], BF16, tag="stg", name=f"stg{e}")
                for h in range(2):
                    abps = pp_ab.tile([128, 4, 256], F32, tag="pab", name=f"ab{e}_{h}")
                    for k in range(4):
                        nc.tensor.matmul(
                            abps[:, k, :],
                            lhsT=eq_sb[e][:, ts(4 * h + k, 128)],
                            rhs=arhs[:, e, :],
                        )
                    # raw [A|Bm]x4 -> bf16; host divides by r
                    if h == 0:
                        nc.scalar.activation(
                            stg[:, 0:1024],
                            abps.rearrange("p k x -> p (k x)"), func=AF.Copy,
                        )
                        nc.sync.dma_start(OUT[e][:, 0:1024], stg[:, 0:1024])
                    else:
                        nc.vector.tensor_copy(
                            stg[:, 1024 : NT * 256].rearrange(
                                "p (k x) -> p k x", k=4
                            ),
                            abps,
                        )
                nc.gpsimd.tensor_copy(stg[:, NT * 256 :], rc_sb[e])
                nc.sync.dma_start(OUT[e][:, 1024:OW], stg[:, 1024:OW])

            # ---- software-pipelined main loop ----
            emit_e1(0)
            emit_e1(1)
            emit_tp(0)
            for e in range(B_LOC):
                if e + 3 < B_LOC:
                    nc.gpsimd.dma_start(ct_sb[e + 3], CT8[e + 3])
                if e + 2 < B_LOC:
                    nc.gpsimd.dma_start(cub_sb[e + 2], CUB8[e + 2])
                emit_traw(e)
                if e + 2 < B_LOC:
                    emit_e1(e + 2)
                if e + 1 < B_LOC:
                    emit_tp(e + 1)
                emit_abm(e)

    nc.compile()
    return nc


_GRAPH = None


def _graph():
    global _GRAPH
    if _GRAPH is None:
        _GRAPH = _build_graph()
    return _GRAPH


def make_in_maps(C, Q, Cmask, Qmask, w_c, w_q, w_mul):
    """Shard full inputs into per-core input maps (host-side layout prep)."""
    C = np.asarray(C, dtype=np.float32)
    Q = np.asarray(Q, dtype=np.float32)
    wmul_col = np.asarray(w_mul, dtype=np.float32).reshape(D)
    wc_col = np.asarray(w_c, dtype=np.float32).reshape(D)
    wq_col = np.asarray(w_q, dtype=np.float32).reshape(D)
    in_maps = []
    for i in range(N_CORES):
        sl = slice(i * B_LOC, (i + 1) * B_LOC)
        Ci = C[sl]
        Qi = Q[sl]
        qneg = (np.asarray(Qmask[sl], dtype=np.float32) - 1.0) * 1e9  # [8, 128]
        cm = np.asarray(Cmask[sl], dtype=np.float32)  # [8, 1024]

        # CT8 [8, 128, 1024]: [e, d, i]
        ct8 = np.ascontiguousarray(Ci.transpose(0, 2, 1).astype(np.float16))
        # CUB8 [8, 128, 8*129]: per e, tile t: [cm*C tile | cm], p-major
        cmC = Ci * cm[:, :, None]
        cub = np.empty((B_LOC, NT, 128, 129), dtype=ml_dtypes.bfloat16)
        cub[:, :, :, 0:128] = cmC.reshape(B_LOC, NT, 128, D)
        cub[:, :, :, 128] = cm.reshape(B_LOC, NT, 128)
        cub8 = np.ascontiguousarray(
            cub.transpose(0, 2, 1, 3).reshape(B_LOC, 128, NT * 129)
        )
        # QTP [128, 8, 128]: [d, e, j]
        qtp = np.ascontiguousarray(Qi.transpose(2, 0, 1).astype(np.float16))
        # QB1 [128, 8, 128]: [j, e, d]
        qb1 = np.ascontiguousarray(Qi.transpose(1, 0, 2).astype(ml_dtypes.bfloat16))
        # CST [128, 12]: wmul | wc | qneg.T | wq | wq
        cst = np.empty((128, 12), dtype=np.float32)
        cst[:, 0] = wmul_col
        cst[:, 1] = wc_col
        cst[:, 2:10] = qneg.T
        cst[:, 10] = wq_col
        cst[:, 11] = wq_col
        in_maps.append(
            {
                "CT8": ct8,
                "CUB8": cub8,
                "QTP": qtp,
                "QB1": qb1,
                "CST": cst,
            }
        )
    return in_maps


def _assemble_core(out_dev, Ci, dst):
    """out_dev [8,128,OW] bf16, Ci [8,1024,128] f32 -> dst [8,1024,512]."""
    v = np.asarray(out_dev).astype(np.float32)
    ab = v[:, :, 0 : NT * 256].reshape(B_LOC, 128, NT, 2, 128)
    r = v[:, :, NT * 256 :].reshape(B_LOC, 128, NT, 1)
    a = (ab[:, :, :, 0, :] / r).transpose(0, 2, 1, 3).reshape(B_LOC, LC, D)
    bm = (ab[:, :, :, 1, :] / r).transpose(0, 2, 1, 3).reshape(B_LOC, LC, D)
    dst[:, :, 0:D] = Ci
    dst[:, :, D : 2 * D] = a
    dst[:, :, 2 * D : 3 * D] = Ci * a
    dst[:, :, 3 * D : 4 * D] = Ci * bm


def assemble(results, C):
    """Gather per-core device outputs + input C into the full f32 output."""
    C = np.asarray(C, dtype=np.float32)
    out = np.empty((B, LC, 4 * D), dtype=np.float32)
    for i in range(N_CORES):
        sl = slice(i * B_LOC, (i + 1) * B_LOC)
        _assemble_core(results[i]["OUT"], C[sl], out[sl])
    return out


def kernel(C, Q, Cmask, Qmask, w_c, w_q, w_mul, bias=None, **_ignored):
    # `bias` is mathematically a no-op: it shifts every score equally and
    # softmax is shift-invariant, so the output does not depend on it.
    nc = _graph()
    in_maps = make_in_maps(C, Q, Cmask, Qmask, w_c, w_q, w_mul)
    res = run_bass_kernel_spmd(nc, in_maps, core_ids=list(range(N_CORES)))
    return assemble(res.results, C)
